# revision 8
# baseline (speedup 1.0000x reference)
"""nn_ConvModel kernel — data-parallel Bass/Tile implementation on 8 NeuronCores.

Strategy (per sharding_hint): shard the batch dim of `image` across 8 cores,
replicate the tiny 3-bit-quantized weights, and turn each per-tensor
fake-quant absmax reduction into a max-AllReduce across the cores.

All heavy math runs in the integer domain (activations/weights are small
integers, exact in bf16; matmuls accumulate exactly in f32 PSUM):
  - linear1 runs on the PE as a block-diagonal stationary matmul producing
    activations in [(l',c) partition, batch] layout,
  - the depthwise K=15 conv runs on the PE as dense block-Toeplitz matmuls
    over l-blocks of 4 (5 accumulating stationaries per output block),
  - the final linear contracts all 84 layout tiles into one PSUM bank.
Each fake_quant is an affine + round-to-nearest-even (magic-constant) +
tanh (ACT engine) pipeline split across DVE/ACT/GPSIMD. Per-tensor scales
are computed on device: local absmax -> gpsimd partition_all_reduce ->
AllReduce(max) collective -> scale arithmetic on [128,1] tiles. Biases are
injected into the matmuls via extra stationary rows (bf16 hi+lo split).

The Bass program is compiled and warmed at import time; kernel() calls the
cached PJRT executable. Falls back to a pure-NumPy implementation if the
device path is unavailable.
"""
import os
import sys

import numpy as np

N_CORES = 8
BATCH = 4096
BC = BATCH // N_CORES   # 512 images per core
NCB = BC // 128         # 4 chunks of 128 images
MODEL_DIM = 384
L = 28                  # sequence length (rows of image)
F = 28                  # features per row (cols of image)
KK = 15                 # conv kernel size
A = 7                   # l-blocks of 4
G = 12                  # channel groups of 32
MAGIC = float(np.float32(12582912.0))  # 1.5 * 2^23; (x+M)-M == rne(x) for |x|<2^22

_M = np.float32(MAGIC)


# ----------------------------------------------------------------------------
# host-side quantization helpers (bit-compatible with the jax reference)
# ----------------------------------------------------------------------------

def _rne(x):
    return (x.astype(np.float32) + _M) - _M


def _scale_np(absmax, bits):
    qmax = np.float32(2 ** (bits - 1) - 1)
    return np.maximum(np.float32(absmax) / qmax, np.float32(1e-8))


def _quant_weight(w, bits):
    qmax = float(2 ** (bits - 1) - 1)
    qmin = -float(2 ** (bits - 1))
    s = _scale_np(np.abs(w).max(), bits)
    q = np.clip(_rne(w / s), qmin, qmax).astype(np.float32)
    return q, s


# ----------------------------------------------------------------------------
# constant packing for the device layout
# partition convention inside a (l-block a, c-group g) tile: p = 32*l' + ci
# covering l = 4a + l', c = 32g + ci.
# ----------------------------------------------------------------------------

def _pack_consts(W1, b1, Wc, bc, Wf, bf):
    import ml_dtypes

    qW1, sW1 = _quant_weight(W1, 3)                    # [384, 28]
    qWc, sWc = _quant_weight(Wc, 3)                    # [384, 1, 15]
    qWc = qWc.reshape(MODEL_DIM, KK)
    qWf, sWf = _quant_weight(Wf, 3)                    # [10, 10752]

    # linear1 stationary: [112, G*128]; rows k = 28*la + f (la in 0..3),
    # col = 128*g + 32*lb + ci
    w1s = np.zeros((112, G * 128), np.float32)
    for g in range(G):
        for lb in range(4):
            # rows 28*lb .. 28*lb+27, cols for all ci
            blk = qW1[32 * g:32 * (g + 1), :]          # [32 ci, 28 f]
            w1s[28 * lb:28 * (lb + 1), 128 * g + 32 * lb: 128 * g + 32 * (lb + 1)] = blk.T
    # conv stationaries: [128, 5*G*128]; block (o, g): rows 32*li+ci,
    # cols 32*lo+co; value qWc[c, d+7] with d = 4*(o-2) + li - lo
    wcs = np.zeros((128, 5 * G * 128), np.float32)
    for o in range(5):
        off = o - 2
        for g in range(G):
            base = (o * G + g) * 128
            for li in range(4):
                for lo in range(4):
                    d = 4 * off + li - lo
                    if -7 <= d <= 7:
                        cis = np.arange(32)
                        wcs[32 * li + cis, base + 32 * lo + cis] = qWc[32 * g + cis, d + 7]
    # final linear moving: [128, A*G*10]; block (a, g): rows 32*lp+ci,
    # cols o; value qWf[o, (4a+lp)*384 + 32g + ci]
    wfm = np.zeros((128, A * G * 10), np.float32)
    for a in range(A):
        for g in range(G):
            base = (a * G + g) * 10
            for lp in range(4):
                for ci in range(32):
                    wfm[32 * lp + ci, base:base + 10] = qWf[:, (4 * a + lp) * 384 + 32 * g + ci]
    # permuted biases pre-divided by the weight scale: [1, G*128],
    # col = 128*g + 32*lp + ci -> b[32g+ci]/sW; the device multiplies by the
    # remaining 1/s* factor in place.
    b1d = (b1.astype(np.float32) / sW1).astype(np.float32)
    bcd = (bc.astype(np.float32) / sWc).astype(np.float32)
    b1p = np.zeros((1, G * 128), np.float32)
    bcp = np.zeros((1, G * 128), np.float32)
    for g in range(G):
        for lp in range(4):
            b1p[0, 128 * g + 32 * lp: 128 * g + 32 * lp + 32] = b1d[32 * g:32 * (g + 1)]
            bcp[0, 128 * g + 32 * lp: 128 * g + 32 * lp + 32] = bcd[32 * g:32 * (g + 1)]
    bfr = (bf.reshape(1, 10).astype(np.float32) / sWf).astype(np.float32)
    sw = np.zeros((128, 3), np.float32)
    sw[:, 0] = sW1
    sw[:, 1] = sWc
    sw[:, 2] = sWf

    bf16 = ml_dtypes.bfloat16
    return dict(
        w1s=w1s.astype(bf16), wcs=wcs.astype(bf16), wfm=wfm.astype(bf16),
        b1p=b1p, bcp=bcp, bfr=bfr, sw=sw,
    )


# ----------------------------------------------------------------------------
# pure-NumPy fallback (the previous baseline implementation)
# ----------------------------------------------------------------------------

def _kernel_numpy(image, W1, b1, Wc, bc, Wf, bf):
    qW1, sW1 = _quant_weight(W1, 3)
    qWc, sWc = _quant_weight(Wc, 3)
    qWf, sWf = _quant_weight(Wf, 3)

    x = np.ascontiguousarray(image.reshape(BATCH, L, F), np.float32)
    s0 = _scale_np(np.abs(x).max(), 8)
    c0 = np.float32(1.0) / s0
    qx = _rne(x * c0)

    raw1 = qx.reshape(-1, F).dot(qW1.T).reshape(BATCH, L, MODEL_DIM)
    k1 = s0 * sW1
    s1 = _scale_np(np.abs(raw1 * k1 + b1).max(), 8)
    ql = _rne(raw1 * (k1 / s1) + b1 / s1)
    s2 = _scale_np(np.tanh(np.float32(127.0) * s1), 8)
    q1 = _rne(np.tanh(s1 * ql, dtype=np.float32) * (np.float32(1.0) / s2))

    k3 = s2 * sWc
    qp = np.pad(q1, ((0, 0), (7, 7), (0, 0)))
    raw3 = np.zeros_like(q1)
    for k in range(KK):
        raw3 += qp[:, k:k + L, :] * qWc[:, 0, k][None, None, :]
    s3 = _scale_np(np.abs(raw3 * k3 + bc[None, None, :]).max(), 8)
    qc = _rne(raw3 * (k3 / s3) + (bc / s3)[None, None, :])
    s4 = _scale_np(np.tanh(np.float32(127.0) * s3), 8)
    q2 = _rne(np.tanh(s3 * qc, dtype=np.float32) * (np.float32(1.0) / s4))

    k5 = s4 * sWf
    qWfT = np.ascontiguousarray(qWf.reshape(10, -1).T)
    logits = q2.reshape(BATCH, -1) @ qWfT * k5 + bf
    s5 = _scale_np(np.abs(logits).max(), 8)
    return (_rne(logits / s5) * s5).astype(np.float32)


# ----------------------------------------------------------------------------
# device path
# ----------------------------------------------------------------------------

_fn = None
_init_err = None


def _build_device_fn():
    """Build + jit the 8-core Bass program. Returns the callable."""
    if os.environ.get("JAX_PLATFORMS") == "cpu":
        # a harness may have forced cpu for the reference; we need the device
        os.environ["JAX_PLATFORMS"] = ""
    import jax
    from jax.sharding import Mesh, PartitionSpec as P

    import concourse.bass as bass  # noqa: F401
    import concourse.tile as tile
    import concourse.bass_isa as bass_isa
    from concourse import mybir, masks
    from concourse.bass2jax import bass_jit, bass_shard_map

    F32 = mybir.dt.float32
    BF16 = mybir.dt.bfloat16
    AL = mybir.AluOpType
    AF = mybir.ActivationFunctionType
    AX = mybir.AxisListType
    RG = [list(range(N_CORES))]

    @bass_jit
    def _core(nc, img, w1s, wcs, wfm, b1p, bcp, bfr, sw):
        out = nc.dram_tensor("out", [BC, 10], F32, kind="ExternalOutput")
        with tile.TileContext(nc) as tc:
            with (
                tc.tile_pool(name="const", bufs=1) as pc,
                tc.tile_pool(name="qxT", bufs=1) as pqxT,
                tc.tile_pool(name="imgp", bufs=4) as pim,
                tc.tile_pool(name="qimp", bufs=4) as pqi,
                tc.tile_pool(name="q1p", bufs=84) as pq1,
                tc.tile_pool(name="q2p", bufs=4) as pq2,
                tc.tile_pool(name="up", bufs=2) as pu,
                tc.tile_pool(name="qp", bufs=2) as pq,
                tc.tile_pool(name="tp", bufs=2) as pt,
                tc.tile_pool(name="wp", bufs=2) as pw,
                tc.tile_pool(name="smal", bufs=1) as psm,
                tc.tile_pool(name="ptr", bufs=2, space="PSUM") as ppt,
                tc.tile_pool(name="pmm", bufs=4, space="PSUM") as ppm,
                tc.tile_pool(name="pout", bufs=1, space="PSUM") as ppo,
                tc.tile_pool(name="dram", bufs=1, space="DRAM") as pdr,
            ):
                # ---- constants into SBUF
                w1s_t = pc.tile([112, G * 128], BF16)
                wcs_t = pc.tile([128, 5 * G * 128], BF16)
                wfm_t = pc.tile([128, A * G * 10], BF16)
                sw_t = pc.tile([128, 3], F32)
                nc.sync.dma_start(w1s_t[:], w1s[:])
                nc.sync.dma_start(wcs_t[:], wcs[:])
                nc.sync.dma_start(wfm_t[:], wfm[:])
                nc.sync.dma_start(sw_t[:], sw[:])

                ident_bf = pc.tile([128, 128], BF16)
                masks.make_identity(nc, ident_bf[:])
                ident = pc.tile([32, 32], F32)
                masks.make_identity(nc, ident[:])
                ones1 = pc.tile([1, 512], F32)
                nc.gpsimd.memset(ones1[:], 1.0)
                b1s_t = pc.tile([1, G * 128], F32)    # b1/sW1, scaled by 1/s0 in place
                bcs_t = pc.tile([1, G * 128], F32)    # bc/sWc, scaled by 1/s2 in place
                bfm_t = pc.tile([1, 10], F32)         # bf/sWf, scaled by 1/s4 in place
                nc.sync.dma_start(b1s_t[:], b1p[:])
                nc.sync.dma_start(bcs_t[:], bcp[:])
                nc.sync.dma_start(bfm_t[:], bfr[:])

                # scalar scratch: one [128, 32] tile, column slices
                scal = psm.tile([128, 32], F32)
                scal2 = psm.tile([10, 8], F32)

                def sc(i):
                    return scal[:, i:i + 1]

                # scalar column indices
                (G0L, G0, S0, C0, K1, RK1, G1, S1, RS1, A1, T2, S2, RS2,
                 K3, RK3, G3, S3, RS3, A3, T4, S4, RS4, K5, RK5,
                 G1L, G3L, TM1, TM3) = range(28)

                # local-max scratch (memset to 0; abs-max is >= 0)
                mx0 = psm.tile([128, 8], F32)
                mx1 = psm.tile([128, 96], F32)
                mx3 = psm.tile([128, 96], F32)
                nc.gpsimd.memset(mx0[:], 0.0)
                nc.gpsimd.memset(mx1[:], 0.0)
                nc.gpsimd.memset(mx3[:], 0.0)

                # collective bounce buffers
                cc_in = [pdr.tile([128, 1], F32, name=f"ccin{i}") for i in range(3)]
                cc_out = [
                    pdr.tile([128, 1], F32, addr_space="Shared", name=f"ccout{i}")
                    for i in range(3)
                ]
                cc_in3 = pdr.tile([10, 1], F32, name="ccin3")
                cc_out3 = pdr.tile([10, 1], F32, addr_space="Shared", name="ccout3")

                def allreduce_max(src_col, dst_col, idx):
                    nc.gpsimd.partition_all_reduce(
                        dst_col, src_col, 128, bass_isa.ReduceOp.max
                    )
                    nc.sync.dma_start(cc_in[idx][:], dst_col)
                    nc.gpsimd.collective_compute(
                        "AllReduce", AL.max,
                        ins=[cc_in[idx][:]], outs=[cc_out[idx][:]],
                        replica_groups=RG,
                    )
                    nc.sync.dma_start(dst_col, cc_out[idx][:])

                # ---- phase 1/2: load image chunks, local absmax, AR, quantize,
                # then bf16 transposes into the (l',f) moving layout
                imt = [pim.tile([128, L * F], F32, name=f"im{cb}") for cb in range(NCB)]
                for cb in range(NCB):
                    nc.sync.dma_start(imt[cb][:], img[128 * cb:128 * (cb + 1), :])
                    nc.vector.tensor_reduce(
                        mx0[:, cb:cb + 1], imt[cb][:], AX.X, AL.max,
                        apply_absolute_value=True,
                    )
                nc.vector.tensor_reduce(sc(G0L), mx0[:], AX.X, AL.max)
                allreduce_max(sc(G0L), sc(G0), 0)

                # stage-0 scales
                nc.vector.tensor_scalar(sc(S0), sc(G0), 1.0 / 127.0, 1e-8, AL.mult, AL.max)
                nc.vector.reciprocal(sc(C0), sc(S0))
                nc.vector.tensor_tensor(sc(K1), sc(S0), sw_t[:, 0:1], AL.mult)

                # linear1 runtime bias stationary: b1s *= 1/s0  (in place)
                nc.vector.tensor_scalar(
                    b1s_t[:], b1s_t[:], scal[0:1, C0:C0 + 1], None, AL.mult
                )

                # quantize image in natural layout: qim = rne(im * c0), bf16
                qim = [pqi.tile([128, L * F], BF16, name=f"qim{cb}") for cb in range(NCB)]
                for cb in range(NCB):
                    tq = pu.tile([128, L * F], F32, tag="tq")
                    nc.vector.tensor_scalar(
                        tq[:], imt[cb][:], sc(C0), MAGIC, AL.mult, AL.add
                    )
                    nc.vector.tensor_scalar(qim[cb][:], tq[:], -MAGIC, None, AL.add)
                # bf16 transposes -> qxT[a] [112, 512]
                qxT = [pqxT.tile([112, 512], BF16, name=f"qxT{a}") for a in range(A)]
                for cb in range(NCB):
                    for a in range(A):
                        ptr = ppt.tile([112, 128], BF16, tag="ptr_bf")
                        nc.tensor.transpose(
                            ptr[:], qim[cb][:, 112 * a:112 * (a + 1)], ident_bf[:]
                        )
                        nc.scalar.activation(
                            qxT[a][:, 128 * cb:128 * (cb + 1)], ptr[:], AF.Copy
                        )

                # ---- mm1 pass 1: absmax of (raw1 + b1/k1)
                idx = 0
                for a in range(A):
                    for g in range(G):
                        ps = ppm.tile([128, 512], F32)
                        nc.tensor.matmul(
                            ps[:], w1s_t[:, 128 * g:128 * (g + 1)], qxT[a][:],
                            start=True, stop=False,
                        )
                        nc.tensor.matmul(
                            ps[:], b1s_t[0:1, 128 * g:128 * (g + 1)], ones1[:],
                            start=False, stop=True,
                        )
                        nc.vector.tensor_reduce(
                            mx1[:, idx:idx + 1], ps[:], AX.X, AL.max,
                            apply_absolute_value=True,
                        )
                        idx += 1
                nc.vector.tensor_reduce(sc(G1L), mx1[:], AX.X, AL.max)
                allreduce_max(sc(G1L), sc(G1), 1)

                # stage-1/2 scales
                nc.vector.tensor_tensor(sc(TM1), sc(G1), sc(K1), AL.mult)
                nc.vector.tensor_scalar(sc(S1), sc(TM1), 1.0 / 127.0, 1e-8, AL.mult, AL.max)
                nc.vector.reciprocal(sc(RS1), sc(S1))
                nc.vector.tensor_tensor(sc(A1), sc(K1), sc(RS1), AL.mult)
                nc.scalar.activation(sc(T2), sc(S1), AF.Tanh, scale=127.0)
                nc.vector.tensor_scalar(sc(S2), sc(T2), 1.0 / 127.0, 1e-8, AL.mult, AL.max)
                nc.vector.reciprocal(sc(RS2), sc(S2))
                nc.vector.tensor_tensor(sc(K3), sc(S2), sw_t[:, 1:2], AL.mult)

                # conv runtime bias stationary: bcs *= 1/s2  (in place)
                nc.vector.tensor_scalar(
                    bcs_t[:], bcs_t[:], scal[0:1, RS2:RS2 + 1], None, AL.mult
                )

                # ---- mm1 pass 2 -> q1 tiles
                q1t = {}
                for a in range(A):
                    for g in range(G):
                        ps = ppm.tile([128, 512], F32)
                        nc.tensor.matmul(
                            ps[:], w1s_t[:, 128 * g:128 * (g + 1)], qxT[a][:],
                            start=True, stop=False,
                        )
                        nc.tensor.matmul(
                            ps[:], b1s_t[0:1, 128 * g:128 * (g + 1)], ones1[:],
                            start=False, stop=True,
                        )
                        u = pu.tile([128, 512], F32)
                        nc.vector.tensor_scalar(
                            u[:], ps[:], sc(A1), MAGIC, AL.mult, AL.add
                        )
                        qs = pq.tile([128, 512], F32)
                        nc.gpsimd.tensor_scalar(
                            qs[:], u[:], -MAGIC, sc(S1), AL.add, AL.mult
                        )
                        tt = pt.tile([128, 512], F32)
                        nc.scalar.activation(tt[:], qs[:], AF.Tanh)
                        ww = pw.tile([128, 512], F32)
                        nc.scalar.activation(
                            ww[:], tt[:], AF.Copy, bias=MAGIC, scale=sc(RS2)
                        )
                        qt = pq1.tile([128, 512], BF16, name=f"q1_{a}_{g}", tag="q1")
                        nc.gpsimd.tensor_scalar(qt[:], ww[:], -MAGIC, None, AL.add)
                        q1t[(a, g)] = qt

                # ---- conv pass 1: absmax of (raw3 + bc/k3)
                def conv_mms(a, g, ps, with_bias):
                    offs = [o for o in (-2, -1, 0, 1, 2) if 0 <= a + o < A]
                    for j, o in enumerate(offs):
                        nc.tensor.matmul(
                            ps[:],
                            wcs_t[:, ((o + 2) * G + g) * 128:((o + 2) * G + g + 1) * 128],
                            q1t[(a + o, g)][:],
                            start=(j == 0), stop=(not with_bias and j == len(offs) - 1),
                        )
                    if with_bias:
                        nc.tensor.matmul(
                            ps[:], bcs_t[0:1, 128 * g:128 * (g + 1)], ones1[:],
                            start=False, stop=True,
                        )

                idx = 0
                for a in range(A):
                    for g in range(G):
                        ps = ppm.tile([128, 512], F32)
                        conv_mms(a, g, ps, True)
                        nc.vector.tensor_reduce(
                            mx3[:, idx:idx + 1], ps[:], AX.X, AL.max,
                            apply_absolute_value=True,
                        )
                        idx += 1
                nc.vector.tensor_reduce(sc(G3L), mx3[:], AX.X, AL.max)
                allreduce_max(sc(G3L), sc(G3), 2)

                # stage-3/4 scales
                nc.vector.tensor_tensor(sc(TM3), sc(G3), sc(K3), AL.mult)
                nc.vector.tensor_scalar(sc(S3), sc(TM3), 1.0 / 127.0, 1e-8, AL.mult, AL.max)
                nc.vector.reciprocal(sc(RS3), sc(S3))
                nc.vector.tensor_tensor(sc(A3), sc(K3), sc(RS3), AL.mult)
                nc.scalar.activation(sc(T4), sc(S3), AF.Tanh, scale=127.0)
                nc.vector.tensor_scalar(sc(S4), sc(T4), 1.0 / 127.0, 1e-8, AL.mult, AL.max)
                nc.vector.reciprocal(sc(RS4), sc(S4))
                nc.vector.tensor_tensor(sc(K5), sc(S4), sw_t[:, 2:3], AL.mult)

                # final bias stationary: bfm *= 1/s4  (in place)
                nc.vector.tensor_scalar(
                    bfm_t[:], bfm_t[:], scal[0:1, RS4:RS4 + 1], None, AL.mult
                )

                # ---- conv pass 2 -> q2 -> mm3 accumulation
                pso = ppo.tile([10, 512], F32)
                first = True
                for a in range(A):
                    for g in range(G):
                        ps = ppm.tile([128, 512], F32)
                        conv_mms(a, g, ps, True)
                        u = pu.tile([128, 512], F32)
                        nc.vector.tensor_scalar(
                            u[:], ps[:], sc(A3), MAGIC, AL.mult, AL.add
                        )
                        qs = pq.tile([128, 512], F32)
                        nc.gpsimd.tensor_scalar(
                            qs[:], u[:], -MAGIC, sc(S3), AL.add, AL.mult
                        )
                        tt = pt.tile([128, 512], F32)
                        nc.scalar.activation(tt[:], qs[:], AF.Tanh)
                        ww = pw.tile([128, 512], F32)
                        nc.scalar.activation(
                            ww[:], tt[:], AF.Copy, bias=MAGIC, scale=sc(RS4)
                        )
                        q2 = pq2.tile([128, 512], BF16)
                        nc.gpsimd.tensor_scalar(q2[:], ww[:], -MAGIC, None, AL.add)
                        nc.tensor.matmul(
                            pso[:], wfm_t[:, (a * G + g) * 10:(a * G + g + 1) * 10],
                            q2[:], start=first, stop=False, skip_group_check=True,
                        )
                        first = False
                nc.tensor.matmul(
                    pso[:], bfm_t[:], ones1[:], start=False, stop=True,
                    skip_group_check=True,
                )

                # ---- final absmax + quantize + output
                nc.vector.tensor_reduce(
                    scal2[:, 0:1], pso[:], AX.X, AL.max, apply_absolute_value=True
                )
                nc.gpsimd.partition_all_reduce(
                    scal2[:, 1:2], scal2[:, 0:1], 10, bass_isa.ReduceOp.max
                )
                nc.sync.dma_start(cc_in3[:], scal2[:, 1:2])
                nc.gpsimd.collective_compute(
                    "AllReduce", AL.max,
                    ins=[cc_in3[:]], outs=[cc_out3[:]], replica_groups=RG,
                )
                nc.sync.dma_start(scal2[:, 1:2], cc_out3[:])

                nc.vector.tensor_tensor(scal2[:, 2:3], scal2[:, 1:2], scal[0:10, K5:K5 + 1], AL.mult)
                nc.vector.tensor_scalar(scal2[:, 3:4], scal2[:, 2:3], 1.0 / 127.0, 1e-8, AL.mult, AL.max)
                nc.vector.reciprocal(scal2[:, 4:5], scal2[:, 3:4])
                nc.vector.tensor_tensor(scal2[:, 5:6], scal[0:10, K5:K5 + 1], scal2[:, 4:5], AL.mult)

                uo = pu.tile([128, 512], F32)
                nc.vector.tensor_scalar(
                    uo[0:10, :], pso[:], scal2[:, 5:6], MAGIC, AL.mult, AL.add
                )
                oq = pw.tile([128, 512], F32)
                nc.vector.tensor_scalar(
                    oq[0:10, :], uo[0:10, :], -MAGIC, scal2[:, 3:4], AL.add, AL.mult
                )
                for cb in range(NCB):
                    ptr = ppt.tile([128, 128], F32)
                    nc.tensor.transpose(
                        ptr[0:128, 0:10], oq[0:10, 128 * cb:128 * (cb + 1)],
                        ident[0:10, 0:10],
                    )
                    fo = pq.tile([128, 512], F32)
                    nc.scalar.activation(fo[:, 0:10], ptr[0:128, 0:10], AF.Copy)
                    nc.sync.dma_start(
                        out[128 * cb:128 * (cb + 1), :], fo[:, 0:10]
                    )
        return out

    devices = jax.devices()[:N_CORES]
    mesh = Mesh(np.asarray(devices), ("core",))
    reps = (P("core"),) + (P(),) * 7
    fn = bass_shard_map(_core, mesh=mesh, in_specs=reps, out_specs=P("core"))
    return fn


def _init():
    global _fn, _init_err
    if _fn is not None or _init_err is not None:
        return
    try:
        fn = _build_device_fn()
        # warm: trace + compile + load with dummy inputs
        z = np.zeros((BATCH, L * F), np.float32)
        c = _pack_consts(
            np.zeros((MODEL_DIM, F), np.float32),
            np.zeros((MODEL_DIM,), np.float32),
            np.zeros((MODEL_DIM, 1, KK), np.float32),
            np.zeros((MODEL_DIM,), np.float32),
            np.zeros((10, MODEL_DIM * L), np.float32),
            np.zeros((10,), np.float32),
        )
        np.asarray(fn(z, c["w1s"], c["wcs"], c["wfm"], c["b1p"], c["bcp"],
                      c["bfr"], c["sw"]))
        _fn = fn
    except Exception as e:  # pragma: no cover
        import traceback
        traceback.print_exc()
        _init_err = e


def kernel(image, W1, b1, Wc, bc, Wf, bf):
    image = np.ascontiguousarray(np.asarray(image, np.float32).reshape(BATCH, L * F))
    W1 = np.asarray(W1, np.float32)
    b1 = np.asarray(b1, np.float32)
    Wc = np.asarray(Wc, np.float32)
    bc = np.asarray(bc, np.float32)
    Wf = np.asarray(Wf, np.float32)
    bf = np.asarray(bf, np.float32)

    _init()
    if _fn is None:
        return _kernel_numpy(image, W1, b1, Wc, bc, Wf, bf)
    try:
        c = _pack_consts(W1, b1, Wc, bc, Wf, bf)
        out = np.asarray(_fn(image, c["w1s"], c["wcs"], c["wfm"], c["b1p"],
                             c["bcp"], c["bfr"], c["sw"]))
        return np.ascontiguousarray(out.astype(np.float32))
    except Exception:  # pragma: no cover
        import traceback
        traceback.print_exc()
        return _kernel_numpy(image, W1, b1, Wc, bc, Wf, bf)


_init()
